# revision 30
# baseline (speedup 1.0000x reference)
"""DGCNN forward on 8 Trainium2 cores (self-contained).

Strategy: 500 graphs (200 nodes, block-diagonal random edges) padded to 512,
64 graphs/core, 16 tiles x 4 graph-lanes. Device computes the 3 hidden GCN
layers with an exact-precision scheme:

    h_{l+1} = tanh( inv (.) ( M @ ( inv (.) h_l @ W ) ) )

where A = D^-1/2 (Adj+I) D^-1/2 is factored into the INTEGER matrix
M = Adj+I (exactly representable in fp16) and the per-node inv = deg^-1/2
scalings (applied on vector/scalar engines in fp32). The transform h@W runs
in fp32 on the PE; its scaled output t' is split into fp16 hi+lo and the
propagate runs as two accumulated fp16 matmuls against M^T — numerically
~fp32 end to end at 1 cycle/row PE speed.

Host does: M/inv construction, x pre-scale+transpose, the 1-channel layer-4
sort key in exact fp32 from the device's fp32 h3, and the SortAggregation +
conv/MLP tail (tiny).
"""
import os
import numpy as np

N_GRAPHS, N_PER, K_TOP, F_IN, H = 500, 200, 30, 128, 32
G_PAD = 512          # padded graph count (8 cores x 64)
G_CORE = 64          # graphs per core
NL = 4               # graph lanes per tile (partition groups of 32)
NT = G_CORE // NL    # tiles per core (16)
C0, C1 = 128, 72     # node chunks per graph (s-dim split for K<=128)


def _build_mi(edge_index):
    """Integer adjacency M = Adj+I per graph [G_PAD,200,200] fp32 + inv=deg^-1/2."""
    n = N_GRAPHS * N_PER
    src = np.concatenate([edge_index[0].astype(np.int64), np.arange(n, dtype=np.int64)])
    dst = np.concatenate([edge_index[1].astype(np.int64), np.arange(n, dtype=np.int64)])
    deg = np.bincount(dst, minlength=n).astype(np.float32)
    inv = (1.0 / np.sqrt(np.maximum(deg, 1e-12))).astype(np.float32)
    M = np.zeros((G_PAD, N_PER, N_PER), np.float32)
    np.add.at(M, (dst // N_PER, dst % N_PER, src % N_PER), np.float32(1))
    invg = np.zeros((G_PAD, N_PER), np.float32)
    invg[:N_GRAPHS] = inv.reshape(N_GRAPHS, N_PER)
    return M, invg


def _host_tail(hcat, inputs):
    """hcat [G, 200, 97] -> output [G, 1], exact fp32 numpy mirror of reference."""
    G = hcat.shape[0]
    order = np.argsort(-hcat[:, :, -1], axis=1, kind="stable")[:, :K_TOP]
    topk = np.take_along_axis(hcat, order[:, :, None], axis=1)      # [G,30,97]
    C1w = np.asarray(inputs["cw1"], np.float32)[:, 0, :].T           # [97,16]
    c1 = np.maximum(np.einsum("gkc,co->gko", topk, C1w) + np.asarray(inputs["cb1"], np.float32), 0)
    p1 = np.maximum(c1[:, 0::2, :], c1[:, 1::2, :])                  # [G,15,16]
    cw2 = np.asarray(inputs["cw2"], np.float32)                      # [32,16,5]
    c2 = np.zeros((G, 11, 32), np.float32)
    for k in range(5):
        c2 += np.einsum("gti,io->gto", p1[:, k:k + 11, :], cw2[:, :, k].T)
    c2 = np.maximum(c2 + np.asarray(inputs["cb2"], np.float32), 0)
    flat = c2.transpose(0, 2, 1).reshape(G, -1)                      # [G,352]
    z = np.maximum(flat @ np.asarray(inputs["lw1"], np.float32) + np.asarray(inputs["lb1"], np.float32), 0)
    o = z @ np.asarray(inputs["lw2"], np.float32) + np.asarray(inputs["lb2"], np.float32)
    return (1.0 / (1.0 + np.exp(-o))).astype(np.float32)


def _device_gcn(xt, mt0, mt1, invb_h, w1, w2b, w3b):
    """Run 3 GCN layers on 8 NeuronCores. All matmul operand reads at
    partition base 0; propagate outputs use PE column quadrants 0/32/64/96.

    xt   [8, NT, 128, NL, 200] fp32  -- pre-scaled x'^T per graph (f-major)
    mt0  [8, NT, 128, NL, 200] fp16  -- M^T rows 0..127
    mt1  [8, NT,  72, NL, 200] fp16  -- M^T rows 128..199
    invb_h [8, NT, 128, 200] fp32    -- inv per graph bcast over 32-feat groups
    w1 [128,32] fp32; w2b/w3b [128,128] fp32 block-diag kron(I4, W)
    Returns hout [8, NT, 128, 2, 200] bf16 (h1,h2), h3out [8, NT, 128, 200] fp32.
    """
    import concourse.bacc as bacc
    import concourse.mybir as mybir
    import concourse.tile as tile
    from concourse import bass_utils

    dt = mybir.dt
    ACT = mybir.ActivationFunctionType
    nc = bacc.Bacc("TRN2", target_bir_lowering=False, debug=False, num_devices=8)

    # xt and invb packed: [..., 0:800] = x'^T lanes, [..., 800:1000] = invb
    d_xi = nc.dram_tensor("xtiv", (NT, 128, NL * 200 + 200), dt.float32,
                          kind="ExternalInput").ap()
    d_m0 = nc.dram_tensor("mt0", (NT, 128, NL, 200), dt.float16, kind="ExternalInput").ap()
    d_m1 = nc.dram_tensor("mt1", (NT, C1, NL, 200), dt.float16, kind="ExternalInput").ap()
    d_w1 = nc.dram_tensor("w1", (128, 32), dt.float32, kind="ExternalInput").ap()
    d_w2 = nc.dram_tensor("w2b", (128, 128), dt.float32, kind="ExternalInput").ap()
    d_w3 = nc.dram_tensor("w3b", (128, 128), dt.float32, kind="ExternalInput").ap()
    d_ho = nc.dram_tensor("hout", (NT, 128, 2, 200), dt.bfloat16, kind="ExternalOutput").ap()
    d_h3 = nc.dram_tensor("h3out", (NT, 128, 200), dt.float32, kind="ExternalOutput").ap()

    CSZ = (C0, C1)
    with tile.TileContext(nc) as tc:
        with tc.tile_pool(name="wp", bufs=1) as wp, \
             tc.tile_pool(name="sb", bufs=3) as sb, \
             tc.tile_pool(name="ps", bufs=2, space="PSUM") as ps:
            w1s = wp.tile([128, 32], dt.float32, name="w1s")
            w2s = wp.tile([128, 128], dt.float32, name="w2s")
            w3s = wp.tile([128, 128], dt.float32, name="w3s")
            wbs = [None, w2s, w3s]
            wload = [False]

            def load_group(ts, sfxs):
                sts = []
                for t, sfx in zip(ts, sfxs):
                    st = {"t": t}
                    xi = sb.tile([128, NL * 200 + 200], dt.float32, tag=f"xt{sfx}", name="xi")
                    nc.sync.dma_start(out=xi[:], in_=d_xi[t])
                    st["xi"] = xi
                    if not wload[0]:
                        nc.sync.dma_start(out=w1s[:], in_=d_w1[:])
                        nc.sync.dma_start(out=w2s[:], in_=d_w2[:])
                        nc.sync.dma_start(out=w3s[:], in_=d_w3[:])
                        wload[0] = True
                    sts.append(st)
                for st, sfx in zip(sts, sfxs):
                    st["m0s"] = sb.tile([128, NL, 200], dt.float16, tag=f"m0{sfx}", name="m0s")
                    nc.sync.dma_start(out=st["m0s"][:], in_=d_m0[st["t"]])
                for st, sfx in zip(sts, sfxs):
                    st["m1s"] = sb.tile([C1, NL, 200], dt.float16, tag=f"m1{sfx}", name="m1s")
                    nc.sync.dma_start(out=st["m1s"][:], in_=d_m1[st["t"]])
                    st["hout"] = sb.tile([128, 2, 200], dt.bfloat16, tag=f"ho{sfx}", name="hout")
                return sts

            def transform(st, l, sfx):
                # px packs transform out (cols 0:256) + propagate out (256:456)
                # into one PSUM bank so 4 pipeline slots double-buffer in 8 banks
                px = ps.tile([128, 456], dt.float32, tag=f"px{sfx}", name="px")
                for c in range(2):
                    cn = CSZ[c]
                    if l == 0:
                        for q in range(NL):
                            off = q * 200 + 128 * c
                            nc.tensor.matmul(px[0:cn, 128 * c + 32 * q:128 * c + 32 * q + 32],
                                             lhsT=st["xi"][:, off:off + cn],
                                             rhs=w1s[:], start=True, stop=True)
                    else:
                        nc.tensor.matmul(px[0:cn, 128 * c:128 * c + 128],
                                         lhsT=st["hcur"][:, 128 * c:128 * c + cn],
                                         rhs=wbs[l][:], start=True, stop=True)
                st["px"] = px

            def split(st, l, sfx):
                tsb = sb.tile([128, 2, NL, 2, 32], dt.float16, tag=f"ts{sfx}", name="tsb")
                for c in range(2):
                    cn = CSZ[c]
                    nc.scalar.activation(tsb[0:cn, c, :, 0, :],
                                         st["px"][0:cn, 128 * c:128 * c + 128], ACT.Copy)
                    nc.vector.tensor_sub(tsb[0:cn, c, :, 1, :],
                                         st["px"][0:cn, 128 * c:128 * c + 128],
                                         tsb[0:cn, c, :, 0, :])
                st["tsb"] = tsb

            def propagate(st, l, sfx):
                px = st["px"]
                for q in range(NL):
                    k = 0
                    for c in range(2):
                        cn = CSZ[c]
                        mts = st["m0s"] if c == 0 else st["m1s"]
                        for hl in range(2):
                            nc.tensor.matmul(px[32 * q:32 * q + 32, 256:456],
                                             lhsT=st["tsb"][0:cn, c, q, hl, :],
                                             rhs=mts[0:cn, q, :],
                                             start=(k == 0), stop=(k == 3),
                                             tile_position=(0, 32 * q))
                            k += 1

            def post(st, l, sfx):
                psc = sb.tile([128, 200], dt.float32, tag=f"ps{sfx}", name="psc")
                nc.vector.tensor_mul(psc[:], st["px"][:, 256:456],
                                     st["xi"][:, NL * 200:NL * 200 + 200])
                hf = sb.tile([128, 200], dt.float32, tag=f"h{l}{sfx}", name=f"h{l}")
                nc.scalar.activation(hf[:], psc[:], ACT.Tanh)
                if l < 2:
                    nc.gpsimd.tensor_copy(st["hout"][:, l, :], hf[:])
                    nc.sync.dma_start(out=d_ho[st["t"]][:, l, :], in_=st["hout"][:, l, :])
                    hp = sb.tile([128, 200], dt.float32, tag=f"hp{l}{sfx}", name=f"hp{l}")
                    nc.vector.tensor_mul(hp[:], hf[:], st["xi"][:, NL * 200:NL * 200 + 200])
                    st["hcur"] = hp
                else:
                    nc.sync.dma_start(out=d_h3[st["t"]], in_=hf[:])

            # G-way software pipeline: interleave stages of a tile group so each
            # engine queue alternates independent work
            G = 4
            SFX = "abcd"[:G]
            for t in range(0, NT, G):
                sts = load_group([t + j for j in range(G)], SFX)
                for l in range(3):
                    for st, sfx in zip(sts, SFX):
                        transform(st, l, sfx)
                    for st, sfx in zip(sts, SFX):
                        split(st, l, sfx)
                    for st, sfx in zip(sts, SFX):
                        propagate(st, l, sfx)
                    for st, sfx in zip(sts, SFX):
                        post(st, l, sfx)


    nc.compile()

    in_maps = [{"xtiv": xt[c], "mt0": mt0[c], "mt1": mt1[c],
                "w1": w1, "w2b": w2b, "w3b": w3b} for c in range(8)]
    trace = bool(int(os.environ.get("BASS_KERNEL_TRACE", "0")))
    if trace:
        try:
            import trace_hook
            trace_hook.install()
        except Exception:
            pass
    res = bass_utils.run_bass_kernel_spmd(nc, in_maps, core_ids=list(range(8)), trace=trace)
    if trace and res.exec_time_ns is not None:
        print(f"HW exec time: {res.exec_time_ns} ns")
    ho = np.stack([res.results[c]["hout"] for c in range(8)])
    h3 = np.stack([res.results[c]["h3out"] for c in range(8)])
    return ho, h3


def kernel(**inputs):
    x = np.asarray(inputs["x"], np.float32)            # [100000, 128]
    ei = np.asarray(inputs["edge_index"])
    M, invg = _build_mi(ei)                             # [512,200,200] fp32, [512,200]
    Ws = [np.asarray(inputs[f"W{i}"], np.float32) for i in (1, 2, 3, 4)]
    bs = [np.asarray(inputs[f"b{i}"], np.float32) for i in (1, 2, 3, 4)]
    xg = x.reshape(N_GRAPHS, N_PER, F_IN)

    use_device = all(np.all(b == 0) for b in bs[:3])
    hcat = None
    if use_device:
        try:
            # ---- host prep ----
            xp = np.zeros((G_PAD, N_PER, F_IN), np.float32)
            xp[:N_GRAPHS] = xg * invg[:N_GRAPHS, :, None]           # pre-scale by inv
            # xt [core, tile, f, lane, n]
            xt = (xp.transpose(0, 2, 1)                              # [G,128,200]
                    .reshape(8, NT, NL, F_IN, N_PER)
                    .transpose(0, 1, 3, 2, 4).copy())
            MT = M.transpose(0, 2, 1)                                # [G, s, d]
            mt = MT.astype(np.float16)                               # integers: exact
            mt0 = (mt[:, 0:C0].reshape(8, NT, NL, C0, N_PER)
                     .transpose(0, 1, 3, 2, 4).copy())
            mt1 = (mt[:, C0:N_PER].reshape(8, NT, NL, C1, N_PER)
                     .transpose(0, 1, 3, 2, 4).copy())
            # invb [core, tile, 32q+f, d] = inv_graph(q)[d], bcast over f
            invb_h = np.repeat(invg.reshape(8, NT, NL, 1, N_PER), 32, axis=3) \
                       .reshape(8, NT, 128, N_PER)
            # pack xt lanes + invb into one DMA stream [core, tile, 128, 1000]
            xtiv = np.concatenate(
                [xt.reshape(8, NT, 128, NL * N_PER), invb_h], axis=3).copy()
            w2b = np.kron(np.eye(NL, dtype=np.float32), Ws[1])       # [128,128]
            w3b = np.kron(np.eye(NL, dtype=np.float32), Ws[2])
            ho, h3o = _device_gcn(xtiv, mt0, mt1, None, Ws[0], w2b, w3b)
            # unpack: [8, NT, 32q+f, l, d] -> h_l [512, 200, 32]
            ho = np.asarray(ho).astype(np.float32)                   # bf16 -> fp32
            ho = (ho.reshape(8, NT, NL, 32, 2, N_PER)
                    .transpose(4, 0, 1, 2, 5, 3).reshape(2, G_PAD, N_PER, 32))
            h3 = (np.asarray(h3o).reshape(8, NT, NL, 32, N_PER)
                    .transpose(0, 1, 2, 4, 3).reshape(G_PAD, N_PER, 32))
            # ---- host layer 4 (1-channel sort key) in exact fp32 ----
            t4 = (h3 @ Ws[3]) * invg[:, :, None]                     # [512,200,1]
            p4 = np.matmul(M, t4) * invg[:, :, None]                 # [512,200,1]
            h4 = np.tanh(p4 + bs[3])
            hcat = np.concatenate(
                [ho[0, :N_GRAPHS], ho[1, :N_GRAPHS], h3[:N_GRAPHS], h4[:N_GRAPHS]],
                axis=-1).astype(np.float32)                          # [500,200,97]
        except Exception as e:
            print("device path failed, falling back to host:", repr(e))
            hcat = None
    if hcat is None:
        A = M[:N_GRAPHS] * invg[:N_GRAPHS, :, None] * invg[:N_GRAPHS, None, :]
        h = xg
        hs = []
        for l in range(4):
            h = np.tanh(np.einsum("gds,gsf->gdf", A, h) @ Ws[l] + bs[l])
            hs.append(h)
        hcat = np.concatenate([hs[0], hs[1], hs[2], hs[3][:, :, :1]], axis=-1)
    return _host_tail(hcat, inputs)


# revision 31
# speedup vs baseline: 1.0538x; 1.0538x over previous
"""DGCNN forward on 8 Trainium2 cores (self-contained).

Strategy: 500 graphs (200 nodes, block-diagonal random edges) padded to 512,
64 graphs/core, 16 tiles x 4 graph-lanes. Device computes the 3 hidden GCN
layers with an exact-precision scheme:

    h_{l+1} = tanh( inv (.) ( M @ ( inv (.) h_l @ W ) ) )

where A = D^-1/2 (Adj+I) D^-1/2 is factored into the INTEGER matrix
M = Adj+I (exactly representable in fp16) and the per-node inv = deg^-1/2
scalings (applied on vector/scalar engines in fp32). The transform h@W runs
in fp32 on the PE; its scaled output t' is split into fp16 hi+lo and the
propagate runs as two accumulated fp16 matmuls against M^T — numerically
~fp32 end to end at 1 cycle/row PE speed.

Host does: M/inv construction, x pre-scale+transpose, the 1-channel layer-4
sort key in exact fp32 from the device's fp32 h3, and the SortAggregation +
conv/MLP tail (tiny).
"""
import os
import numpy as np

N_GRAPHS, N_PER, K_TOP, F_IN, H = 500, 200, 30, 128, 32
G_PAD = 512          # padded graph count (8 cores x 64)
G_CORE = 64          # graphs per core
NL = 4               # graph lanes per tile (partition groups of 32)
NT = G_CORE // NL    # tiles per core (16)
C0, C1 = 128, 72     # node chunks per graph (s-dim split for K<=128)


def _build_mi(edge_index):
    """Integer adjacency M = Adj+I per graph [G_PAD,200,200] fp32 + inv=deg^-1/2."""
    n = N_GRAPHS * N_PER
    src = np.concatenate([edge_index[0].astype(np.int64), np.arange(n, dtype=np.int64)])
    dst = np.concatenate([edge_index[1].astype(np.int64), np.arange(n, dtype=np.int64)])
    deg = np.bincount(dst, minlength=n).astype(np.float32)
    inv = (1.0 / np.sqrt(np.maximum(deg, 1e-12))).astype(np.float32)
    M = np.zeros((G_PAD, N_PER, N_PER), np.float32)
    np.add.at(M, (dst // N_PER, dst % N_PER, src % N_PER), np.float32(1))
    invg = np.zeros((G_PAD, N_PER), np.float32)
    invg[:N_GRAPHS] = inv.reshape(N_GRAPHS, N_PER)
    return M, invg


def _host_tail(hcat, inputs):
    """hcat [G, 200, 97] -> output [G, 1], exact fp32 numpy mirror of reference."""
    G = hcat.shape[0]
    order = np.argsort(-hcat[:, :, -1], axis=1, kind="stable")[:, :K_TOP]
    topk = np.take_along_axis(hcat, order[:, :, None], axis=1)      # [G,30,97]
    C1w = np.asarray(inputs["cw1"], np.float32)[:, 0, :].T           # [97,16]
    c1 = np.maximum(np.einsum("gkc,co->gko", topk, C1w) + np.asarray(inputs["cb1"], np.float32), 0)
    p1 = np.maximum(c1[:, 0::2, :], c1[:, 1::2, :])                  # [G,15,16]
    cw2 = np.asarray(inputs["cw2"], np.float32)                      # [32,16,5]
    c2 = np.zeros((G, 11, 32), np.float32)
    for k in range(5):
        c2 += np.einsum("gti,io->gto", p1[:, k:k + 11, :], cw2[:, :, k].T)
    c2 = np.maximum(c2 + np.asarray(inputs["cb2"], np.float32), 0)
    flat = c2.transpose(0, 2, 1).reshape(G, -1)                      # [G,352]
    z = np.maximum(flat @ np.asarray(inputs["lw1"], np.float32) + np.asarray(inputs["lb1"], np.float32), 0)
    o = z @ np.asarray(inputs["lw2"], np.float32) + np.asarray(inputs["lb2"], np.float32)
    return (1.0 / (1.0 + np.exp(-o))).astype(np.float32)


def _device_gcn(xt, mt0, mt1, invb_h, w1, w2b, w3b):
    """Run 3 GCN layers on 8 NeuronCores. All matmul operand reads at
    partition base 0; propagate outputs use PE column quadrants 0/32/64/96.

    xt   [8, NT, 128, NL, 200] fp32  -- pre-scaled x'^T per graph (f-major)
    mt0  [8, NT, 128, NL, 200] fp16  -- M^T rows 0..127
    mt1  [8, NT,  72, NL, 200] fp16  -- M^T rows 128..199
    invb_h [8, NT, 128, 200] fp32    -- inv per graph bcast over 32-feat groups
    w1 [128,32] fp32; w2b/w3b [128,128] fp32 block-diag kron(I4, W)
    Returns hout [8, NT, 128, 2, 200] bf16 (h1,h2), h3out [8, NT, 128, 200] fp32.
    """
    import concourse.bacc as bacc
    import concourse.mybir as mybir
    import concourse.tile as tile
    from concourse import bass_utils

    dt = mybir.dt
    ACT = mybir.ActivationFunctionType
    nc = bacc.Bacc("TRN2", target_bir_lowering=False, debug=False, num_devices=8)

    # xt and invb packed: [..., 0:800] = x'^T lanes, [..., 800:1000] = invb
    d_xi = nc.dram_tensor("xtiv", (NT, 128, NL * 200 + 200), dt.float32,
                          kind="ExternalInput").ap()
    d_m0 = nc.dram_tensor("mt0", (NT, 128, NL, 200), dt.float16, kind="ExternalInput").ap()
    d_m1 = nc.dram_tensor("mt1", (NT, C1, NL, 200), dt.float16, kind="ExternalInput").ap()
    d_w1 = nc.dram_tensor("w1", (128, 32), dt.float32, kind="ExternalInput").ap()
    d_w2 = nc.dram_tensor("w2b", (128, 128), dt.float32, kind="ExternalInput").ap()
    d_w3 = nc.dram_tensor("w3b", (128, 128), dt.float32, kind="ExternalInput").ap()
    d_ho = nc.dram_tensor("hout", (NT, 128, 2, 200), dt.bfloat16, kind="ExternalOutput").ap()
    d_h3 = nc.dram_tensor("h3out", (NT, 128, 200), dt.float32, kind="ExternalOutput").ap()

    CSZ = (C0, C1)
    with tile.TileContext(nc) as tc:
        with tc.tile_pool(name="wp", bufs=1) as wp, \
             tc.tile_pool(name="sb", bufs=3) as sb, \
             tc.tile_pool(name="ps", bufs=1, space="PSUM") as ps:
            w1s = wp.tile([128, 32], dt.float32, name="w1s")
            w2s = wp.tile([128, 128], dt.float32, name="w2s")
            w3s = wp.tile([128, 128], dt.float32, name="w3s")
            wbs = [None, w2s, w3s]
            wload = [False]

            def load_group(ts, sfxs):
                sts = []
                for t, sfx in zip(ts, sfxs):
                    st = {"t": t}
                    xi = sb.tile([128, NL * 200 + 200], dt.float32, tag=f"xt{sfx}", name="xi")
                    nc.sync.dma_start(out=xi[:], in_=d_xi[t])
                    st["xi"] = xi
                    if not wload[0]:
                        nc.sync.dma_start(out=w1s[:], in_=d_w1[:])
                        nc.sync.dma_start(out=w2s[:], in_=d_w2[:])
                        nc.sync.dma_start(out=w3s[:], in_=d_w3[:])
                        wload[0] = True
                    sts.append(st)
                for st, sfx in zip(sts, sfxs):
                    st["m0s"] = sb.tile([128, NL, 200], dt.float16, tag=f"m0{sfx}", name="m0s")
                    nc.sync.dma_start(out=st["m0s"][:], in_=d_m0[st["t"]])
                for st, sfx in zip(sts, sfxs):
                    st["m1s"] = sb.tile([C1, NL, 200], dt.float16, tag=f"m1{sfx}", name="m1s")
                    nc.sync.dma_start(out=st["m1s"][:], in_=d_m1[st["t"]])
                    st["hout"] = sb.tile([128, 2, 200], dt.bfloat16, tag=f"ho{sfx}", name="hout")
                return sts

            def transform(st, l, sfx):
                tp = ps.tile([128, 2, NL, 32], dt.float32, tag=f"tp{sfx}", name="tp")
                for c in range(2):
                    cn = CSZ[c]
                    if l == 0:
                        for q in range(NL):
                            off = q * 200 + 128 * c
                            nc.tensor.matmul(tp[0:cn, c, q, :],
                                             lhsT=st["xi"][:, off:off + cn],
                                             rhs=w1s[:], start=True, stop=True)
                    else:
                        nc.tensor.matmul(tp[0:cn, c, :, :],
                                         lhsT=st["hcur"][:, 128 * c:128 * c + cn],
                                         rhs=wbs[l][:], start=True, stop=True)
                st["tp"] = tp

            def split(st, l, sfx):
                tsb = sb.tile([128, 2, NL, 2, 32], dt.float16, tag=f"ts{sfx}", name="tsb")
                for c in range(2):
                    cn = CSZ[c]
                    nc.scalar.activation(tsb[0:cn, c, :, 0, :], st["tp"][0:cn, c, :, :], ACT.Copy)
                    nc.vector.tensor_sub(tsb[0:cn, c, :, 1, :], st["tp"][0:cn, c, :, :],
                                         tsb[0:cn, c, :, 0, :])
                st["tsb"] = tsb

            def propagate(st, l, sfx):
                pp = ps.tile([128, 200], dt.float32, tag=f"pp{sfx}", name="pp")
                for q in range(NL):
                    k = 0
                    for c in range(2):
                        cn = CSZ[c]
                        mts = st["m0s"] if c == 0 else st["m1s"]
                        for hl in range(2):
                            nc.tensor.matmul(pp[32 * q:32 * q + 32, :],
                                             lhsT=st["tsb"][0:cn, c, q, hl, :],
                                             rhs=mts[0:cn, q, :],
                                             start=(k == 0), stop=(k == 3),
                                             tile_position=(0, 32 * q))
                            k += 1
                st["pp"] = pp

            def post(st, l, sfx):
                psc = sb.tile([128, 200], dt.float32, tag=f"ps{sfx}", name="psc")
                nc.vector.tensor_mul(psc[:], st["pp"][:],
                                     st["xi"][:, NL * 200:NL * 200 + 200])
                hf = sb.tile([128, 200], dt.float32, tag=f"h{l}{sfx}", name=f"h{l}")
                nc.scalar.activation(hf[:], psc[:], ACT.Tanh)
                if l < 2:
                    nc.gpsimd.tensor_copy(st["hout"][:, l, :], hf[:])
                    nc.sync.dma_start(out=d_ho[st["t"]][:, l, :], in_=st["hout"][:, l, :])
                    hp = sb.tile([128, 200], dt.float32, tag=f"hp{l}{sfx}", name=f"hp{l}")
                    nc.vector.tensor_mul(hp[:], hf[:], st["xi"][:, NL * 200:NL * 200 + 200])
                    st["hcur"] = hp
                else:
                    nc.sync.dma_start(out=d_h3[st["t"]], in_=hf[:])

            # G-way software pipeline: interleave stages of a tile group so each
            # engine queue alternates independent work
            G = 4
            SFX = "abcd"[:G]
            for t in range(0, NT, G):
                sts = load_group([t + j for j in range(G)], SFX)
                for l in range(3):
                    for st, sfx in zip(sts, SFX):
                        transform(st, l, sfx)
                    for st, sfx in zip(sts, SFX):
                        split(st, l, sfx)
                    for st, sfx in zip(sts, SFX):
                        propagate(st, l, sfx)
                    for st, sfx in zip(sts, SFX):
                        post(st, l, sfx)


    nc.compile()

    in_maps = [{"xtiv": xt[c], "mt0": mt0[c], "mt1": mt1[c],
                "w1": w1, "w2b": w2b, "w3b": w3b} for c in range(8)]
    trace = bool(int(os.environ.get("BASS_KERNEL_TRACE", "0")))
    if trace:
        try:
            import trace_hook
            trace_hook.install()
        except Exception:
            pass
    res = bass_utils.run_bass_kernel_spmd(nc, in_maps, core_ids=list(range(8)), trace=trace)
    if trace and res.exec_time_ns is not None:
        print(f"HW exec time: {res.exec_time_ns} ns")
    ho = np.stack([res.results[c]["hout"] for c in range(8)])
    h3 = np.stack([res.results[c]["h3out"] for c in range(8)])
    return ho, h3


def kernel(**inputs):
    x = np.asarray(inputs["x"], np.float32)            # [100000, 128]
    ei = np.asarray(inputs["edge_index"])
    M, invg = _build_mi(ei)                             # [512,200,200] fp32, [512,200]
    Ws = [np.asarray(inputs[f"W{i}"], np.float32) for i in (1, 2, 3, 4)]
    bs = [np.asarray(inputs[f"b{i}"], np.float32) for i in (1, 2, 3, 4)]
    xg = x.reshape(N_GRAPHS, N_PER, F_IN)

    use_device = all(np.all(b == 0) for b in bs[:3])
    hcat = None
    if use_device:
        try:
            # ---- host prep ----
            xp = np.zeros((G_PAD, N_PER, F_IN), np.float32)
            xp[:N_GRAPHS] = xg * invg[:N_GRAPHS, :, None]           # pre-scale by inv
            # xt [core, tile, f, lane, n]
            xt = (xp.transpose(0, 2, 1)                              # [G,128,200]
                    .reshape(8, NT, NL, F_IN, N_PER)
                    .transpose(0, 1, 3, 2, 4).copy())
            MT = M.transpose(0, 2, 1)                                # [G, s, d]
            mt = MT.astype(np.float16)                               # integers: exact
            mt0 = (mt[:, 0:C0].reshape(8, NT, NL, C0, N_PER)
                     .transpose(0, 1, 3, 2, 4).copy())
            mt1 = (mt[:, C0:N_PER].reshape(8, NT, NL, C1, N_PER)
                     .transpose(0, 1, 3, 2, 4).copy())
            # invb [core, tile, 32q+f, d] = inv_graph(q)[d], bcast over f
            invb_h = np.repeat(invg.reshape(8, NT, NL, 1, N_PER), 32, axis=3) \
                       .reshape(8, NT, 128, N_PER)
            # pack xt lanes + invb into one DMA stream [core, tile, 128, 1000]
            xtiv = np.concatenate(
                [xt.reshape(8, NT, 128, NL * N_PER), invb_h], axis=3).copy()
            w2b = np.kron(np.eye(NL, dtype=np.float32), Ws[1])       # [128,128]
            w3b = np.kron(np.eye(NL, dtype=np.float32), Ws[2])
            ho, h3o = _device_gcn(xtiv, mt0, mt1, None, Ws[0], w2b, w3b)
            # unpack: [8, NT, 32q+f, l, d] -> h_l [512, 200, 32]
            ho = np.asarray(ho).astype(np.float32)                   # bf16 -> fp32
            ho = (ho.reshape(8, NT, NL, 32, 2, N_PER)
                    .transpose(4, 0, 1, 2, 5, 3).reshape(2, G_PAD, N_PER, 32))
            h3 = (np.asarray(h3o).reshape(8, NT, NL, 32, N_PER)
                    .transpose(0, 1, 2, 4, 3).reshape(G_PAD, N_PER, 32))
            # ---- host layer 4 (1-channel sort key) in exact fp32 ----
            t4 = (h3 @ Ws[3]) * invg[:, :, None]                     # [512,200,1]
            p4 = np.matmul(M, t4) * invg[:, :, None]                 # [512,200,1]
            h4 = np.tanh(p4 + bs[3])
            hcat = np.concatenate(
                [ho[0, :N_GRAPHS], ho[1, :N_GRAPHS], h3[:N_GRAPHS], h4[:N_GRAPHS]],
                axis=-1).astype(np.float32)                          # [500,200,97]
        except Exception as e:
            print("device path failed, falling back to host:", repr(e))
            hcat = None
    if hcat is None:
        A = M[:N_GRAPHS] * invg[:N_GRAPHS, :, None] * invg[:N_GRAPHS, None, :]
        h = xg
        hs = []
        for l in range(4):
            h = np.tanh(np.einsum("gds,gsf->gdf", A, h) @ Ws[l] + bs[l])
            hs.append(h)
        hcat = np.concatenate([hs[0], hs[1], hs[2], hs[3][:, :, :1]], axis=-1)
    return _host_tail(hcat, inputs)


# revision 32
# speedup vs baseline: 1.0709x; 1.0162x over previous
"""DGCNN forward on 8 Trainium2 cores (self-contained).

Strategy: 500 graphs (200 nodes, block-diagonal random edges) padded to 512,
64 graphs/core, 16 tiles x 4 graph-lanes. Device computes the 3 hidden GCN
layers with an exact-precision scheme:

    h_{l+1} = tanh( inv (.) ( M @ ( inv (.) h_l @ W ) ) )

where A = D^-1/2 (Adj+I) D^-1/2 is factored into the INTEGER matrix
M = Adj+I (exactly representable in fp16) and the per-node inv = deg^-1/2
scalings (applied on vector/scalar engines in fp32). The transform h@W runs
in fp32 on the PE; its scaled output t' is split into fp16 hi+lo and the
propagate runs as two accumulated fp16 matmuls against M^T — numerically
~fp32 end to end at 1 cycle/row PE speed.

Host does: M/inv construction, x pre-scale+transpose, the 1-channel layer-4
sort key in exact fp32 from the device's fp32 h3, and the SortAggregation +
conv/MLP tail (tiny).
"""
import os
import numpy as np

N_GRAPHS, N_PER, K_TOP, F_IN, H = 500, 200, 30, 128, 32
G_PAD = 512          # padded graph count (8 cores x 64)
G_CORE = 64          # graphs per core
NL = 4               # graph lanes per tile (partition groups of 32)
NT = G_CORE // NL    # tiles per core (16)
C0, C1 = 128, 72     # node chunks per graph (s-dim split for K<=128)


def _build_mi(edge_index):
    """Integer adjacency M = Adj+I per graph [G_PAD,200,200] fp32 + inv=deg^-1/2."""
    n = N_GRAPHS * N_PER
    src = np.concatenate([edge_index[0].astype(np.int64), np.arange(n, dtype=np.int64)])
    dst = np.concatenate([edge_index[1].astype(np.int64), np.arange(n, dtype=np.int64)])
    deg = np.bincount(dst, minlength=n).astype(np.float32)
    inv = (1.0 / np.sqrt(np.maximum(deg, 1e-12))).astype(np.float32)
    M = np.zeros((G_PAD, N_PER, N_PER), np.float32)
    np.add.at(M, (dst // N_PER, dst % N_PER, src % N_PER), np.float32(1))
    invg = np.zeros((G_PAD, N_PER), np.float32)
    invg[:N_GRAPHS] = inv.reshape(N_GRAPHS, N_PER)
    return M, invg


def _host_tail(hcat, inputs):
    """hcat [G, 200, 97] -> output [G, 1], exact fp32 numpy mirror of reference."""
    G = hcat.shape[0]
    order = np.argsort(-hcat[:, :, -1], axis=1, kind="stable")[:, :K_TOP]
    topk = np.take_along_axis(hcat, order[:, :, None], axis=1)      # [G,30,97]
    C1w = np.asarray(inputs["cw1"], np.float32)[:, 0, :].T           # [97,16]
    c1 = np.maximum(np.einsum("gkc,co->gko", topk, C1w) + np.asarray(inputs["cb1"], np.float32), 0)
    p1 = np.maximum(c1[:, 0::2, :], c1[:, 1::2, :])                  # [G,15,16]
    cw2 = np.asarray(inputs["cw2"], np.float32)                      # [32,16,5]
    c2 = np.zeros((G, 11, 32), np.float32)
    for k in range(5):
        c2 += np.einsum("gti,io->gto", p1[:, k:k + 11, :], cw2[:, :, k].T)
    c2 = np.maximum(c2 + np.asarray(inputs["cb2"], np.float32), 0)
    flat = c2.transpose(0, 2, 1).reshape(G, -1)                      # [G,352]
    z = np.maximum(flat @ np.asarray(inputs["lw1"], np.float32) + np.asarray(inputs["lb1"], np.float32), 0)
    o = z @ np.asarray(inputs["lw2"], np.float32) + np.asarray(inputs["lb2"], np.float32)
    return (1.0 / (1.0 + np.exp(-o))).astype(np.float32)


def _device_gcn(xt, mt0, mt1, invb_h, w1, w2b, w3b):
    """Run 3 GCN layers on 8 NeuronCores. All matmul operand reads at
    partition base 0; propagate outputs use PE column quadrants 0/32/64/96.

    xt   [8, NT, 128, NL, 200] fp32  -- pre-scaled x'^T per graph (f-major)
    mt0  [8, NT, 128, NL, 200] fp16  -- M^T rows 0..127
    mt1  [8, NT,  72, NL, 200] fp16  -- M^T rows 128..199
    invb_h [8, NT, 128, 200] fp32    -- inv per graph bcast over 32-feat groups
    w1 [128,32] fp32; w2b/w3b [128,128] fp32 block-diag kron(I4, W)
    Returns hout [8, NT, 128, 2, 200] bf16 (h1,h2), h3out [8, NT, 128, 200] fp32.
    """
    import concourse.bacc as bacc
    import concourse.mybir as mybir
    import concourse.tile as tile
    from concourse import bass_utils

    dt = mybir.dt
    ACT = mybir.ActivationFunctionType
    nc = bacc.Bacc("TRN2", target_bir_lowering=False, debug=False, num_devices=8)

    # xt and invb packed: [..., 0:800] = x'^T lanes, [..., 800:1000] = invb
    d_xi = nc.dram_tensor("xtiv", (NT, 128, NL * 200 + 200), dt.float32,
                          kind="ExternalInput").ap()
    d_m0 = nc.dram_tensor("mt0", (NT, 128, NL, 200), dt.float16, kind="ExternalInput").ap()
    d_m1 = nc.dram_tensor("mt1", (NT, C1, NL, 200), dt.float16, kind="ExternalInput").ap()
    d_w1 = nc.dram_tensor("w1", (128, 32), dt.float32, kind="ExternalInput").ap()
    d_w2 = nc.dram_tensor("w2b", (128, 128), dt.float32, kind="ExternalInput").ap()
    d_w3 = nc.dram_tensor("w3b", (128, 128), dt.float32, kind="ExternalInput").ap()
    d_ho = nc.dram_tensor("hout", (NT, 128, 2, 200), dt.bfloat16, kind="ExternalOutput").ap()
    d_h3 = nc.dram_tensor("h3out", (NT, 128, 200), dt.float32, kind="ExternalOutput").ap()

    CSZ = (C0, C1)
    with tile.TileContext(nc) as tc:
        with tc.tile_pool(name="wp", bufs=1) as wp, \
             tc.tile_pool(name="sb", bufs=3) as sb, \
             tc.tile_pool(name="ps", bufs=1, space="PSUM") as ps:
            w1s = wp.tile([128, 32], dt.float32, name="w1s")
            w2s = wp.tile([128, 128], dt.float32, name="w2s")
            w3s = wp.tile([128, 128], dt.float32, name="w3s")
            wbs = [None, w2s, w3s]
            wload = [False]

            def load_group(ts, sfxs):
                sts = []
                for t, sfx in zip(ts, sfxs):
                    st = {"t": t}
                    xi = sb.tile([128, NL * 200 + 200], dt.float32, tag=f"xt{sfx}", name="xi")
                    nc.sync.dma_start(out=xi[:], in_=d_xi[t])
                    st["xi"] = xi
                    if not wload[0]:
                        nc.sync.dma_start(out=w1s[:], in_=d_w1[:])
                        nc.sync.dma_start(out=w2s[:], in_=d_w2[:])
                        nc.sync.dma_start(out=w3s[:], in_=d_w3[:])
                        wload[0] = True
                    sts.append(st)
                for st, sfx in zip(sts, sfxs):
                    st["m0s"] = sb.tile([128, NL, 200], dt.float16, tag=f"m0{sfx}", name="m0s")
                    nc.sync.dma_start(out=st["m0s"][:], in_=d_m0[st["t"]])
                for st, sfx in zip(sts, sfxs):
                    st["m1s"] = sb.tile([C1, NL, 200], dt.float16, tag=f"m1{sfx}", name="m1s")
                    nc.sync.dma_start(out=st["m1s"][:], in_=d_m1[st["t"]])
                    st["hout"] = sb.tile([128, 2, 200], dt.bfloat16, tag=f"ho{sfx}", name="hout")
                return sts

            def transform(st, l, sfx):
                tp = ps.tile([128, 2, NL, 32], dt.float32, tag=f"tp{sfx}", name="tp")
                for c in range(2):
                    cn = CSZ[c]
                    if l == 0:
                        for q in range(NL):
                            off = q * 200 + 128 * c
                            nc.tensor.matmul(tp[0:cn, c, q, :],
                                             lhsT=st["xi"][:, off:off + cn],
                                             rhs=w1s[:], start=True, stop=True)
                    else:
                        nc.tensor.matmul(tp[0:cn, c, :, :],
                                         lhsT=st["hcur"][:, 128 * c:128 * c + cn],
                                         rhs=wbs[l][:], start=True, stop=True)
                st["tp"] = tp

            def split(st, l, sfx):
                tsb = sb.tile([128, 2, NL, 2, 32], dt.float16, tag=f"ts{sfx}", name="tsb")
                for c in range(2):
                    cn = CSZ[c]
                    nc.scalar.activation(tsb[0:cn, c, :, 0, :], st["tp"][0:cn, c, :, :], ACT.Copy)
                    nc.vector.tensor_sub(tsb[0:cn, c, :, 1, :], st["tp"][0:cn, c, :, :],
                                         tsb[0:cn, c, :, 0, :])
                st["tsb"] = tsb

            def propagate(st, l, sfx):
                pp = ps.tile([128, 200], dt.float32, tag=f"pp{sfx}", name="pp")
                for q in range(NL):
                    k = 0
                    for c in range(2):
                        cn = CSZ[c]
                        mts = st["m0s"] if c == 0 else st["m1s"]
                        for hl in range(2):
                            nc.tensor.matmul(pp[32 * q:32 * q + 32, :],
                                             lhsT=st["tsb"][0:cn, c, q, hl, :],
                                             rhs=mts[0:cn, q, :],
                                             start=(k == 0), stop=(k == 3),
                                             tile_position=(0, 32 * q))
                            k += 1
                st["pp"] = pp

            def post(st, l, sfx):
                psc = sb.tile([128, 200], dt.float32, tag=f"ps{sfx}", name="psc")
                nc.vector.tensor_mul(psc[:], st["pp"][:],
                                     st["xi"][:, NL * 200:NL * 200 + 200])
                hf = sb.tile([128, 200], dt.float32, tag=f"h{l}{sfx}", name=f"h{l}")
                nc.scalar.activation(hf[:], psc[:], ACT.Tanh)
                if l < 2:
                    nc.gpsimd.tensor_copy(st["hout"][:, l, :], hf[:])
                    nc.sync.dma_start(out=d_ho[st["t"]][:, l, :], in_=st["hout"][:, l, :])
                    hp = sb.tile([128, 200], dt.float32, tag=f"hp{l}{sfx}", name=f"hp{l}")
                    nc.vector.tensor_mul(hp[:], hf[:], st["xi"][:, NL * 200:NL * 200 + 200])
                    st["hcur"] = hp
                else:
                    nc.sync.dma_start(out=d_h3[st["t"]], in_=hf[:])

            # G-way software pipeline: interleave stages of a tile group so each
            # engine queue alternates independent work
            G = 4
            SFX = "abcd"[:G]
            for t in range(0, NT, G):
                sts = load_group([t + j for j in range(G)], SFX)
                for l in range(3):
                    for st, sfx in zip(sts, SFX):
                        transform(st, l, sfx)
                    for st, sfx in zip(sts, SFX):
                        split(st, l, sfx)
                    for st, sfx in zip(sts, SFX):
                        propagate(st, l, sfx)
                    for st, sfx in zip(sts, SFX):
                        post(st, l, sfx)


    nc.compile()

    in_maps = [{"xtiv": xt[c], "mt0": mt0[c], "mt1": mt1[c],
                "w1": w1, "w2b": w2b, "w3b": w3b} for c in range(8)]
    trace = bool(int(os.environ.get("BASS_KERNEL_TRACE", "0")))
    if trace:
        try:
            import trace_hook
            trace_hook.install()
        except Exception:
            pass
    try:
        res = bass_utils.run_bass_kernel_spmd(nc, in_maps, core_ids=list(range(8)),
                                              trace=trace)
    except Exception:
        if not trace:
            raise
        res = bass_utils.run_bass_kernel_spmd(nc, in_maps, core_ids=list(range(8)),
                                              trace=False)
    if trace and res.exec_time_ns is not None:
        print(f"HW exec time: {res.exec_time_ns} ns")
    ho = np.stack([res.results[c]["hout"] for c in range(8)])
    h3 = np.stack([res.results[c]["h3out"] for c in range(8)])
    return ho, h3


def kernel(**inputs):
    x = np.asarray(inputs["x"], np.float32)            # [100000, 128]
    ei = np.asarray(inputs["edge_index"])
    M, invg = _build_mi(ei)                             # [512,200,200] fp32, [512,200]
    Ws = [np.asarray(inputs[f"W{i}"], np.float32) for i in (1, 2, 3, 4)]
    bs = [np.asarray(inputs[f"b{i}"], np.float32) for i in (1, 2, 3, 4)]
    xg = x.reshape(N_GRAPHS, N_PER, F_IN)

    use_device = all(np.all(b == 0) for b in bs[:3])
    hcat = None
    if use_device:
        try:
            # ---- host prep ----
            xp = np.zeros((G_PAD, N_PER, F_IN), np.float32)
            xp[:N_GRAPHS] = xg * invg[:N_GRAPHS, :, None]           # pre-scale by inv
            # xt [core, tile, f, lane, n]
            xt = (xp.transpose(0, 2, 1)                              # [G,128,200]
                    .reshape(8, NT, NL, F_IN, N_PER)
                    .transpose(0, 1, 3, 2, 4).copy())
            MT = M.transpose(0, 2, 1)                                # [G, s, d]
            mt = MT.astype(np.float16)                               # integers: exact
            mt0 = (mt[:, 0:C0].reshape(8, NT, NL, C0, N_PER)
                     .transpose(0, 1, 3, 2, 4).copy())
            mt1 = (mt[:, C0:N_PER].reshape(8, NT, NL, C1, N_PER)
                     .transpose(0, 1, 3, 2, 4).copy())
            # invb [core, tile, 32q+f, d] = inv_graph(q)[d], bcast over f
            invb_h = np.repeat(invg.reshape(8, NT, NL, 1, N_PER), 32, axis=3) \
                       .reshape(8, NT, 128, N_PER)
            # pack xt lanes + invb into one DMA stream [core, tile, 128, 1000]
            xtiv = np.concatenate(
                [xt.reshape(8, NT, 128, NL * N_PER), invb_h], axis=3).copy()
            w2b = np.kron(np.eye(NL, dtype=np.float32), Ws[1])       # [128,128]
            w3b = np.kron(np.eye(NL, dtype=np.float32), Ws[2])
            ho, h3o = _device_gcn(xtiv, mt0, mt1, None, Ws[0], w2b, w3b)
            # unpack: [8, NT, 32q+f, l, d] -> h_l [512, 200, 32]
            ho = np.asarray(ho).astype(np.float32)                   # bf16 -> fp32
            ho = (ho.reshape(8, NT, NL, 32, 2, N_PER)
                    .transpose(4, 0, 1, 2, 5, 3).reshape(2, G_PAD, N_PER, 32))
            h3 = (np.asarray(h3o).reshape(8, NT, NL, 32, N_PER)
                    .transpose(0, 1, 2, 4, 3).reshape(G_PAD, N_PER, 32))
            # ---- host layer 4 (1-channel sort key) in exact fp32 ----
            t4 = (h3 @ Ws[3]) * invg[:, :, None]                     # [512,200,1]
            p4 = np.matmul(M, t4) * invg[:, :, None]                 # [512,200,1]
            h4 = np.tanh(p4 + bs[3])
            hcat = np.concatenate(
                [ho[0, :N_GRAPHS], ho[1, :N_GRAPHS], h3[:N_GRAPHS], h4[:N_GRAPHS]],
                axis=-1).astype(np.float32)                          # [500,200,97]
        except Exception as e:
            print("device path failed, falling back to host:", repr(e))
            hcat = None
    if hcat is None:
        A = M[:N_GRAPHS] * invg[:N_GRAPHS, :, None] * invg[:N_GRAPHS, None, :]
        h = xg
        hs = []
        for l in range(4):
            h = np.tanh(np.einsum("gds,gsf->gdf", A, h) @ Ws[l] + bs[l])
            hs.append(h)
        hcat = np.concatenate([hs[0], hs[1], hs[2], hs[3][:, :, :1]], axis=-1)
    return _host_tail(hcat, inputs)


# revision 36
# speedup vs baseline: 1.0929x; 1.0205x over previous
"""DGCNN forward on 8 Trainium2 cores (self-contained).

Strategy: 500 graphs (200 nodes, block-diagonal random edges) padded to 512,
64 graphs/core, 16 tiles x 4 graph-lanes. Device computes the 3 hidden GCN
layers with an exact-precision scheme:

    h_{l+1} = tanh( inv (.) ( M @ ( inv (.) h_l @ W ) ) )

where A = D^-1/2 (Adj+I) D^-1/2 is factored into the INTEGER matrix
M = Adj+I (exactly representable in fp16) and the per-node inv = deg^-1/2
scalings (applied on vector/scalar engines in fp32). The transform h@W runs
in fp32 on the PE; its scaled output t' is split into fp16 hi+lo and the
propagate runs as two accumulated fp16 matmuls against M^T — numerically
~fp32 end to end at 1 cycle/row PE speed.

Host does: M/inv construction, x pre-scale+transpose, the 1-channel layer-4
sort key in exact fp32 from the device's fp32 h3, and the SortAggregation +
conv/MLP tail (tiny).
"""
import os
import numpy as np

N_GRAPHS, N_PER, K_TOP, F_IN, H = 500, 200, 30, 128, 32
G_PAD = 512          # padded graph count (8 cores x 64)
G_CORE = 64          # graphs per core
NL = 4               # graph lanes per tile (partition groups of 32)
NT = G_CORE // NL    # tiles per core (16)
C0, C1 = 128, 72     # node chunks per graph (s-dim split for K<=128)


def _build_mi(edge_index):
    """Integer adjacency M = Adj+I per graph [G_PAD,200,200] fp32 + inv=deg^-1/2."""
    n = N_GRAPHS * N_PER
    src = np.concatenate([edge_index[0].astype(np.int64), np.arange(n, dtype=np.int64)])
    dst = np.concatenate([edge_index[1].astype(np.int64), np.arange(n, dtype=np.int64)])
    deg = np.bincount(dst, minlength=n).astype(np.float32)
    inv = (1.0 / np.sqrt(np.maximum(deg, 1e-12))).astype(np.float32)
    M = np.zeros((G_PAD, N_PER, N_PER), np.float32)
    np.add.at(M, (dst // N_PER, dst % N_PER, src % N_PER), np.float32(1))
    invg = np.zeros((G_PAD, N_PER), np.float32)
    invg[:N_GRAPHS] = inv.reshape(N_GRAPHS, N_PER)
    return M, invg


def _host_tail(hcat, inputs):
    """hcat [G, 200, 97] -> output [G, 1], exact fp32 numpy mirror of reference."""
    G = hcat.shape[0]
    order = np.argsort(-hcat[:, :, -1], axis=1, kind="stable")[:, :K_TOP]
    topk = np.take_along_axis(hcat, order[:, :, None], axis=1)      # [G,30,97]
    C1w = np.asarray(inputs["cw1"], np.float32)[:, 0, :].T           # [97,16]
    c1 = np.maximum(np.einsum("gkc,co->gko", topk, C1w) + np.asarray(inputs["cb1"], np.float32), 0)
    p1 = np.maximum(c1[:, 0::2, :], c1[:, 1::2, :])                  # [G,15,16]
    cw2 = np.asarray(inputs["cw2"], np.float32)                      # [32,16,5]
    c2 = np.zeros((G, 11, 32), np.float32)
    for k in range(5):
        c2 += np.einsum("gti,io->gto", p1[:, k:k + 11, :], cw2[:, :, k].T)
    c2 = np.maximum(c2 + np.asarray(inputs["cb2"], np.float32), 0)
    flat = c2.transpose(0, 2, 1).reshape(G, -1)                      # [G,352]
    z = np.maximum(flat @ np.asarray(inputs["lw1"], np.float32) + np.asarray(inputs["lb1"], np.float32), 0)
    o = z @ np.asarray(inputs["lw2"], np.float32) + np.asarray(inputs["lb2"], np.float32)
    return (1.0 / (1.0 + np.exp(-o))).astype(np.float32)


def _device_gcn(xt, mt0, mt1, invb_h, w1, w2b, w3b):
    """Run 3 GCN layers on 8 NeuronCores. All matmul operand reads at
    partition base 0; propagate outputs use PE column quadrants 0/32/64/96.

    xt   [8, NT, 128, NL, 200] fp32  -- pre-scaled x'^T per graph (f-major)
    mt0  [8, NT, 128, NL, 200] fp16  -- M^T rows 0..127
    mt1  [8, NT,  72, NL, 200] fp16  -- M^T rows 128..199
    invb_h [8, NT, 128, 200] fp32    -- inv per graph bcast over 32-feat groups
    w1 [128,32] fp32; w2b/w3b [128,128] fp32 block-diag kron(I4, W)
    Returns hout [8, NT, 128, 2, 200] bf16 (h1,h2), h3out [8, NT, 128, 200] fp32.
    """
    import concourse.bacc as bacc
    import concourse.mybir as mybir
    import concourse.tile as tile
    from concourse import bass_utils

    dt = mybir.dt
    ACT = mybir.ActivationFunctionType
    nc = bacc.Bacc("TRN2", target_bir_lowering=False, debug=False, num_devices=8)

    # xt and invb packed: [..., 0:800] = x'^T lanes, [..., 800:1000] = invb
    d_xi = nc.dram_tensor("xtiv", (NT, 128, NL * 200 + 200), dt.float32,
                          kind="ExternalInput").ap()
    d_m0 = nc.dram_tensor("mt0", (NT, 128, NL, 200), dt.float16, kind="ExternalInput").ap()
    d_m1 = nc.dram_tensor("mt1", (NT, C1, NL, 200), dt.float16, kind="ExternalInput").ap()
    d_w1 = nc.dram_tensor("w1", (128, 32), dt.float32, kind="ExternalInput").ap()
    d_w2 = nc.dram_tensor("w2b", (128, 128), dt.float32, kind="ExternalInput").ap()
    d_w3 = nc.dram_tensor("w3b", (128, 128), dt.float32, kind="ExternalInput").ap()
    d_ho = nc.dram_tensor("hout", (NT, 128, 2, 200), dt.bfloat16, kind="ExternalOutput").ap()
    d_h3 = nc.dram_tensor("h3out", (NT, 128, 200), dt.float32, kind="ExternalOutput").ap()

    CSZ = (C0, C1)
    with tile.TileContext(nc) as tc:
        with tc.tile_pool(name="wp", bufs=1) as wp, \
             tc.tile_pool(name="sb", bufs=3) as sb, \
             tc.tile_pool(name="ps", bufs=1, space="PSUM") as ps:
            w1s = wp.tile([128, 32], dt.float32, name="w1s")
            w2s = wp.tile([128, 128], dt.float32, name="w2s")
            w3s = wp.tile([128, 128], dt.float32, name="w3s")
            wbs = [None, w2s, w3s]
            wload = [False]

            def load_group(ts, sfxs):
                sts = []
                for t, sfx in zip(ts, sfxs):
                    st = {"t": t}
                    xi = sb.tile([128, NL * 200 + 200], dt.float32, tag=f"xt{sfx}", name="xi")
                    nc.sync.dma_start(out=xi[:], in_=d_xi[t])
                    st["xi"] = xi
                    if not wload[0]:
                        nc.sync.dma_start(out=w1s[:], in_=d_w1[:])
                        nc.sync.dma_start(out=w2s[:], in_=d_w2[:])
                        nc.sync.dma_start(out=w3s[:], in_=d_w3[:])
                        wload[0] = True
                    sts.append(st)
                for st, sfx in zip(sts, sfxs):
                    st["m0s"] = sb.tile([128, NL, 200], dt.float16, tag=f"m0{sfx}", name="m0s")
                    nc.sync.dma_start(out=st["m0s"][:], in_=d_m0[st["t"]])
                for st, sfx in zip(sts, sfxs):
                    st["m1s"] = sb.tile([C1, NL, 200], dt.float16, tag=f"m1{sfx}", name="m1s")
                    nc.sync.dma_start(out=st["m1s"][:], in_=d_m1[st["t"]])
                    st["hout"] = sb.tile([128, 2, 200], dt.bfloat16, tag=f"ho{sfx}", name="hout")
                return sts

            def transform(st, l, sfx):
                tp = ps.tile([128, 2, NL, 32], dt.float32, tag=f"tp{sfx}", name="tp")
                for c in range(2):
                    cn = CSZ[c]
                    if l == 0:
                        for q in range(NL):
                            off = q * 200 + 128 * c
                            nc.tensor.matmul(tp[0:cn, c, q, :],
                                             lhsT=st["xi"][:, off:off + cn],
                                             rhs=w1s[:], start=True, stop=True)
                    else:
                        nc.tensor.matmul(tp[0:cn, c, :, :],
                                         lhsT=st["hcur"][:, 128 * c:128 * c + cn],
                                         rhs=wbs[l][:], start=True, stop=True)
                st["tp"] = tp

            def split(st, l, sfx):
                tsb = sb.tile([128, 2, NL, 2, 32], dt.float16, tag=f"ts{sfx}", name="tsb")
                for c in range(2):
                    cn = CSZ[c]
                    nc.scalar.activation(tsb[0:cn, c, :, 0, :], st["tp"][0:cn, c, :, :], ACT.Copy)
                    nc.vector.tensor_sub(tsb[0:cn, c, :, 1, :], st["tp"][0:cn, c, :, :],
                                         tsb[0:cn, c, :, 0, :])
                st["tsb"] = tsb

            def propagate(st, l, sfx):
                pp = ps.tile([128, 200], dt.float32, tag=f"pp{sfx}", name="pp")
                for q in range(NL):
                    k = 0
                    for c in range(2):
                        cn = CSZ[c]
                        mts = st["m0s"] if c == 0 else st["m1s"]
                        for hl in range(2):
                            nc.tensor.matmul(pp[32 * q:32 * q + 32, :],
                                             lhsT=st["tsb"][0:cn, c, q, hl, :],
                                             rhs=mts[0:cn, q, :],
                                             start=(k == 0), stop=(k == 3),
                                             tile_position=(0, 32 * q))
                            k += 1
                st["pp"] = pp

            def post(st, l, sfx):
                psc = sb.tile([128, 200], dt.float32, tag=f"ps{sfx}", name="psc")
                nc.vector.tensor_mul(psc[:], st["pp"][:],
                                     st["xi"][:, NL * 200:NL * 200 + 200])
                hf = sb.tile([128, 200], dt.float32, tag=f"h{l}{sfx}", name=f"h{l}")
                nc.scalar.activation(hf[:], psc[:], ACT.Tanh)
                if l < 2:
                    nc.gpsimd.tensor_copy(st["hout"][:, l, :], hf[:])
                    nc.sync.dma_start(out=d_ho[st["t"]][:, l, :], in_=st["hout"][:, l, :])
                    hp = sb.tile([128, 200], dt.float32, tag=f"hp{l}{sfx}", name=f"hp{l}")
                    nc.vector.tensor_mul(hp[:], hf[:], st["xi"][:, NL * 200:NL * 200 + 200])
                    st["hcur"] = hp
                else:
                    nc.sync.dma_start(out=d_h3[st["t"]], in_=hf[:])

            # G-way software pipeline: interleave stages of a tile group so each
            # engine queue alternates independent work
            G = 4
            SFX = "abcd"[:G]
            for t in range(0, NT, G):
                sts = load_group([t + j for j in range(G)], SFX)
                for l in range(3):
                    for st, sfx in zip(sts, SFX):
                        transform(st, l, sfx)
                    for st, sfx in zip(sts, SFX):
                        split(st, l, sfx)
                    for st, sfx in zip(sts, SFX):
                        propagate(st, l, sfx)
                    for st, sfx in zip(sts, SFX):
                        post(st, l, sfx)


    nc.compile()

    in_maps = [{"xtiv": xt[c], "mt0": mt0[c], "mt1": mt1[c],
                "w1": w1, "w2b": w2b, "w3b": w3b} for c in range(8)]
    trace = bool(int(os.environ.get("BASS_KERNEL_TRACE", "0")))
    if trace:
        try:
            import trace_hook
            trace_hook.install()
        except Exception:
            pass
    try:
        res = bass_utils.run_bass_kernel_spmd(nc, in_maps, core_ids=list(range(8)),
                                              trace=trace)
    except Exception:
        if not trace:
            raise
        res = bass_utils.run_bass_kernel_spmd(nc, in_maps, core_ids=list(range(8)),
                                              trace=False)
    if trace and res.exec_time_ns is not None:
        print(f"HW exec time: {res.exec_time_ns} ns")
    ho = np.stack([res.results[c]["hout"] for c in range(8)])
    h3 = np.stack([res.results[c]["h3out"] for c in range(8)])
    return ho, h3


def kernel(**inputs):
    x = np.asarray(inputs["x"], np.float32)            # [100000, 128]
    ei = np.asarray(inputs["edge_index"])
    M, invg = _build_mi(ei)                             # [512,200,200] fp32, [512,200]
    Ws = [np.asarray(inputs[f"W{i}"], np.float32) for i in (1, 2, 3, 4)]
    bs = [np.asarray(inputs[f"b{i}"], np.float32) for i in (1, 2, 3, 4)]
    xg = x.reshape(N_GRAPHS, N_PER, F_IN)

    use_device = all(np.all(b == 0) for b in bs[:3])
    hcat = None
    if use_device:
        try:
            # ---- host prep ----
            xp = np.zeros((G_PAD, N_PER, F_IN), np.float32)
            xp[:N_GRAPHS] = xg * invg[:N_GRAPHS, :, None]           # pre-scale by inv
            # xt [core, tile, f, lane, n]
            xt = (xp.transpose(0, 2, 1)                              # [G,128,200]
                    .reshape(8, NT, NL, F_IN, N_PER)
                    .transpose(0, 1, 3, 2, 4).copy())
            MT = M.transpose(0, 2, 1)                                # [G, s, d]
            mt = MT.astype(np.float16)                               # integers: exact
            mt0 = (mt[:, 0:C0].reshape(8, NT, NL, C0, N_PER)
                     .transpose(0, 1, 3, 2, 4).copy())
            mt1 = (mt[:, C0:N_PER].reshape(8, NT, NL, C1, N_PER)
                     .transpose(0, 1, 3, 2, 4).copy())
            # invb [core, tile, 32q+f, d] = inv_graph(q)[d], bcast over f
            invb_h = np.repeat(invg.reshape(8, NT, NL, 1, N_PER), 32, axis=3) \
                       .reshape(8, NT, 128, N_PER)
            # pack xt lanes + invb into one DMA stream [core, tile, 128, 1000]
            xtiv = np.concatenate(
                [xt.reshape(8, NT, 128, NL * N_PER), invb_h], axis=3).copy()
            w2b = np.kron(np.eye(NL, dtype=np.float32), Ws[1])       # [128,128]
            w3b = np.kron(np.eye(NL, dtype=np.float32), Ws[2])
            ho, h3o = _device_gcn(xtiv, mt0, mt1, None, Ws[0], w2b, w3b)
            # unpack: [8, NT, 32q+f, l, d] -> h_l [512, 200, 32]
            ho = np.asarray(ho).astype(np.float32)                   # bf16 -> fp32
            ho = (ho.reshape(8, NT, NL, 32, 2, N_PER)
                    .transpose(4, 0, 1, 2, 5, 3).reshape(2, G_PAD, N_PER, 32))
            h3 = (np.asarray(h3o).reshape(8, NT, NL, 32, N_PER)
                    .transpose(0, 1, 2, 4, 3).reshape(G_PAD, N_PER, 32))
            # ---- host layer 4 (1-channel sort key) in exact fp32 ----
            t4 = (h3 @ Ws[3]) * invg[:, :, None]                     # [512,200,1]
            p4 = np.matmul(M, t4) * invg[:, :, None]                 # [512,200,1]
            h4 = np.tanh(p4 + bs[3])
            hcat = np.concatenate(
                [ho[0, :N_GRAPHS], ho[1, :N_GRAPHS], h3[:N_GRAPHS], h4[:N_GRAPHS]],
                axis=-1).astype(np.float32)                          # [500,200,97]
        except Exception as e:
            print("device path failed, falling back to host:", repr(e))
            hcat = None
    if hcat is None:
        A = M[:N_GRAPHS] * invg[:N_GRAPHS, :, None] * invg[:N_GRAPHS, None, :]
        h = xg
        hs = []
        for l in range(4):
            h = np.tanh(np.einsum("gds,gsf->gdf", A, h) @ Ws[l] + bs[l])
            hs.append(h)
        hcat = np.concatenate([hs[0], hs[1], hs[2], hs[3][:, :, :1]], axis=-1)
    return _host_tail(hcat, inputs)
